# revision 33
# baseline (speedup 1.0000x reference)
"""GCN autoencoder (6x gcn_layer) on 8 TRN2 NeuronCores — fp8 DoubleRow.

Strategy (v2):
  - Rows of adj_/X sharded across 8 cores; weights replicated.
  - adj stored fp8e4 (host-scaled x8192, values in [0,1)): the FULL 8MB
    shard is SBUF-resident (no per-layer streaming), and the adj-mm runs
    in DoubleRow perf mode (2 fp8 k-chunks per matmul, ~1.4x bf16 rate).
  - H (= z @ W) is cast to fp8e4 per layer with a power-of-2 scale beta_l
    folded into the replicated weights host-side; activations apply
    scale 1/(8192*beta_l) so z is true-scale bf16 every layer.
  - fp8 H rounding leaves a column-sum bias that the adjacency averaging
    cannot suppress; it is removed exactly via dcol = colsum(Hq) -
    colsum(H), computed on device with FD=1 matmuls (ones / negated
    z-colsum moving operands), AllReduce'd, and folded into the existing
    activation's per-partition bias operand: relu(s*psz - 0.5*s*dcol).
  - adj-mm produces zT = (adj_shard @ H)^T so the next layer's XW matmul
    consumes it directly; XW matmuls stay bf16 (z bf16 x W bf16).
  - Two row-phases of 512 rows each per layer; after a phase: XW(l+1) ->
    fp8 -> DRAM bounce -> AllGather (half the bytes of bf16) -> next
    layer's H chunks, consumed in arrival-wave order.
  - Layer 1's H1 = X @ W1 computed fully on every core from the
    replicated input X -> no collective before the first adj-mm.
"""

import sys

import numpy as np

if "/opt/trn_rl_repo" not in sys.path:
    sys.path.insert(0, "/opt/trn_rl_repo")

import ml_dtypes

import concourse.bacc as bacc
import concourse.tile as tile
from concourse import mybir
from concourse.bass_utils import run_bass_kernel_spmd

N = 8192
D_IN = 512
NCORES = 8
R = N // NCORES  # 1024 rows per core
DIMS = [(512, 256), (256, 256), (256, 128), (128, 256), (256, 256), (256, 512)]

ASCALE = 8192.0
BETA = [4.0, 1024.0, 4096.0, 16384.0, 65536.0, 262144.0]
SL = [1.0 / (ASCALE * b) for b in BETA]

BF16 = mybir.dt.bfloat16
F32 = mybir.dt.float32
FP8 = mybir.dt.float8e4
NP_BF16 = ml_dtypes.bfloat16
NP_FP8 = ml_dtypes.float8_e4m3
RELU = mybir.ActivationFunctionType.Relu
DR = mybir.MatmulPerfMode.DoubleRow

KO = N // 128  # 64 k-chunks over the gather dim
KP = KO // 2  # 32 DoubleRow k-pairs
RT = R // 128  # 8 local row tiles
NPH = 2
PH = R // NPH  # 512 rows per phase
HALF = RT // NPH  # 4 chunks each core contributes per phase

_CACHED = {}


def _build():
    nc = bacc.Bacc(
        "TRN2",
        target_bir_lowering=False,
        debug=False,
        enable_asserts=False,
        num_devices=NCORES,
    )

    adjT = nc.dram_tensor("adjT", [N, R], FP8, kind="ExternalInput")
    xT = nc.dram_tensor("xT", [D_IN, N], BF16, kind="ExternalInput")
    w_dram = [
        nc.dram_tensor(f"W{i + 1}", list(DIMS[i]), BF16, kind="ExternalInput")
        for i in range(6)
    ]
    # negated column-sum of X (bf16), for layer 1's dcol correction
    csxn = nc.dram_tensor("csxn", [D_IN, 1], BF16, kind="ExternalInput")
    outT = nc.dram_tensor("outT", [DIMS[-1][1], R], F32, kind="ExternalOutput")

    adjT_r = adjT.ap().rearrange("(ko p) r -> p ko r", p=128)
    xT_r = xT.ap().rearrange("(kx p) c -> p kx c", p=128)

    with tile.TileContext(nc) as tc:
        with (
            tc.tile_pool(name="adjres", bufs=1) as adjres_p,
            tc.tile_pool(name="wp", bufs=1) as wp,
            tc.tile_pool(name="xtp", bufs=3) as xtp,
            tc.tile_pool(name="ztp", bufs=8) as ztp,
            tc.tile_pool(name="hp", bufs=6) as hp,
            tc.tile_pool(name="hstage", bufs=6) as hstage,
            tc.tile_pool(name="ostage", bufs=2) as ostage,
            tc.tile_pool(name="czp", bufs=6) as czp,
            tc.tile_pool(name="dcs", bufs=6) as dcs_p,
            tc.tile_pool(name="biasp", bufs=6) as biasp,
            tc.tile_pool(name="psz", bufs=6, space="PSUM") as psz,
            tc.tile_pool(name="psh", bufs=2, space="PSUM") as psh,
            tc.tile_pool(name="dram", bufs=1, space="DRAM") as dram,
        ):
            # ---- resident weights + small constants ----
            # only W1 is loaded up front; W2..W6 wait until after the adj
            # DMAs are queued so XW1's xt stream owns the queues at t=0
            w_sb = []
            for i, (di, do) in enumerate(DIMS):
                w_t = wp.tile([128, di // 128, do], BF16, name=f"w{i}_sb")
                if i == 0:
                    nc.sync.dma_start(
                        w_t[:],
                        w_dram[i].ap().rearrange("(kx p) n -> p kx n", p=128),
                    )
                w_sb.append(w_t)
            ones_sb = wp.tile([128, 1], FP8, name="ones_sb")
            nc.vector.memset(ones_sb[:], 1.0)
            csxn_sb = wp.tile([128, D_IN // 128, 1], BF16, name="csxn_sb")
            nc.sync.dma_start(
                csxn_sb[:], csxn.ap().rearrange("(kx p) one -> p kx one", p=128)
            )

            # warmup AllGather: absorb the collective-stream first-use cost
            # while the CC queue is otherwise idle (overlaps XW1 / barrier)
            for wi, wrows in enumerate((16,)):
                wu_in = dram.tile([wrows, 256], BF16, tag=f"wu{wi}i",
                                  name=f"wu{wi}i")
                wu_out = dram.tile([NCORES * wrows, 256], BF16,
                                   addr_space="Shared", tag=f"wu{wi}o",
                                   name=f"wu{wi}o")
                nc.gpsimd.collective_compute(
                    "AllGather",
                    mybir.AluOpType.bypass,
                    ins=[wu_in[:].opt()],
                    outs=[wu_out[:].opt()],
                    replica_groups=[list(range(NCORES))],
                )

            # ---- layer 1: H1 = X @ W1 computed fully on every core ----
            # quartered: [128, 16, 256] x4; read of chunk g -> quarter g//16
            h1 = [
                hp.tile([128, 16, DIMS[0][1]], FP8, tag="h", name=f"h1_{q}")
                for q in range(4)
            ]
            # dcol1 accumulator row: colsum over all rows of H1q, built by
            # ones-stationary matmuls lagged 2 chunks behind the fp8 casts
            # (so the PE never waits on the just-issued DVE copy)
            d1t = psz.tile([128, PH], F32, tag="psz", name="d1t")

            def emit_dcol1(g):
                nc.tensor.matmul(
                    d1t[0:1, 0:256],
                    ones_sb[:],
                    h1[g // 16][:, g % 16, :],
                    start=(g == 0),
                    stop=False,
                )

            def xw1_chunk(g, xt_t, c):
                ps_h = psh.tile([128, DIMS[0][1]], F32, tag="psh")
                for kx in range(D_IN // 128):
                    nc.tensor.matmul(
                        ps_h[:],
                        xt_t[:, kx, c : c + 128],
                        w_sb[0][:, kx, :],
                        start=(kx == 0),
                        stop=(kx == D_IN // 128 - 1),
                    )
                nc.vector.tensor_copy(h1[g // 16][:, g % 16, :], ps_h[:])
                if g >= 2:
                    emit_dcol1(g - 2)

            # first 4 chunks via single-chunk DMAs (fast first landing),
            # then 4-chunk batches
            for g in range(4):
                xt_t = xtp.tile([128, D_IN // 128, 128], BF16, tag="xts")
                nc.sync.dma_start(xt_t[:], xT_r[:, :, g * 128 : g * 128 + 128])
                xw1_chunk(g, xt_t, 0)
            for g0 in range(4, KO, 4):
                xt_t = xtp.tile([128, D_IN // 128, 512], BF16, tag="xt")
                nc.sync.dma_start(xt_t[:], xT_r[:, :, g0 * 128 : g0 * 128 + 512])
                for g in range(g0, g0 + 4):
                    xw1_chunk(g, xt_t, (g - g0) * 128)

            def h1_read(m, g0):
                return h1[g0 // 16][:, g0 % 16 : g0 % 16 + 2,
                                    m * 128 : (m + 1) * 128]

            h_read = h1_read

            # full resident adj shard (fp8), k-ordered to match consumption
            adj_res = [
                adjres_p.tile([128, 16, R], FP8, name=f"adj_res{q}")
                for q in range(4)
            ]
            for q in range(4):
                for j in range(0, 16, 4):
                    nc.sync.dma_start(
                        adj_res[q][:, j : j + 4, :],
                        adjT_r[:, q * 16 + j : q * 16 + j + 4, 0:R],
                    )
            for i in range(1, 6):
                nc.sync.dma_start(
                    w_sb[i][:],
                    w_dram[i].ap().rearrange("(kx p) n -> p kx n", p=128),
                )
            # a second tiny warmup collective just before layer 1's first
            # real gather: re-aligns the cores (launch skew) so the first
            # H gather doesn't eat the whole stagger
            wu2_in = dram.tile([16, 256], BF16, tag="wu2i", name="wu2i")
            wu2_out = dram.tile([NCORES * 16, 256], BF16, addr_space="Shared",
                                tag="wu2o", name="wu2o")
            nc.gpsimd.collective_compute(
                "AllGather",
                mybir.AluOpType.bypass,
                ins=[wu2_in[:].opt()],
                outs=[wu2_out[:].opt()],
                replica_groups=[list(range(NCORES))],
            )

            def adj_pair(g0, n):
                return adj_res[g0 // 16][:, g0 % 16 : g0 % 16 + 2,
                                         n * PH : (n + 1) * PH]

            # layer 1 dcol tail: flush lagged matmuls, subtract colsum(H1)
            # = -csxn@W1 (csxn pre-negated on host), then bounce the [1,256]
            # row through DRAM to transpose it into a per-partition bias
            mt1 = DIMS[0][1] // 128
            emit_dcol1(KO - 2)
            emit_dcol1(KO - 1)
            for kx in range(D_IN // 128):
                nc.tensor.matmul(
                    d1t[0:1, 0:256],
                    csxn_sb[:, kx, :],
                    w_sb[0][:, kx, :],
                    start=False,
                    stop=(kx == D_IN // 128 - 1),
                )
            d1row = dcs_p.tile([1, DIMS[0][1]], F32, tag="dcr", name="d1row")
            nc.vector.tensor_copy(d1row[:], d1t[0:1, 0:256])
            d1b = dram.tile([DIMS[0][1], 1], F32, tag="d1b", name="d1b")
            nc.sync.dma_start(d1b[:], d1row[:])
            braw1 = biasp.tile([128, mt1], F32, tag="bias", name="braw1")
            nc.sync.dma_start(
                braw1[:],
                d1b.rearrange("(m p) one -> p (m one)", p=128),
            )
            bias_cur = biasp.tile([128, mt1], F32, tag="bias", name="bias1")
            nc.vector.tensor_scalar_mul(bias_cur[:], braw1[:], -0.5 * SL[0])

            # consumption waves in DoubleRow PAIRS (g0 = even chunk index):
            # layer 1 in production order; layers >=2 by producer phase
            pwaves_l1 = [list(range(0, KO // 2, 2)), list(range(KO // 2, KO, 2))]
            pwaves_g = [
                [c * RT + n * HALF + j0
                 for c in range(NCORES) for j0 in (0, 2)]
                for n in range(NPH)
            ]

            for li, (di, do) in enumerate(DIMS):
                last = li == len(DIMS) - 1
                mt = do // 128
                kwaves = pwaves_l1 if li == 0 else pwaves_g

                if not last:
                    di2, do2 = DIMS[li + 1]
                    kxn2 = di2 // 128  # == mt
                    mt2 = do2 // 128
                    # per-wave (and per-column-half for do2=512) H buffers:
                    # h_next[ci][w] holds chunks {c*8 + w*4 + j} at pos c*4+j
                    ncs = 1 if do2 <= 256 else 2
                    dc2 = do2 if do2 <= 256 else 256
                    h_next = [
                        [hp.tile([128, KO // 2, dc2], FP8, tag="h",
                                 name=f"h{li + 2}_{ci}_{w}")
                         for w in range(NPH)]
                        for ci in range(ncs)
                    ]

                    def make_reader(h_tiles, split):
                        def rd(m, g0):
                            ci, mc = (m // 2, m % 2) if split else (0, m)
                            c, r8 = g0 // 8, g0 % 8
                            w, j0 = r8 // 4, r8 % 4
                            return h_tiles[ci][w][:, c * 4 + j0 : c * 4 + j0 + 2,
                                                  mc * 128 : (mc + 1) * 128]
                        return rd

                    tl_t = dcs_p.tile([128, mt2, NCORES], F32, tag="tl",
                                      name=f"tl{li}")

                # L6 needs 8 concurrent psum groups for merged emission; its
                # epilogue never touches the psh pool, so borrow 2 slots
                ps_zs = [
                    [psz.tile([128, PH], F32, tag="psz", name=f"psz{n}_{m}")
                     if n * mt + m < 6 else
                     psh.tile([128, PH], F32, tag="psh", name=f"psz{n}_{m}")
                     for m in range(mt)]
                    for n in range(NPH)
                ]
                mm_cnt = [[0] * mt for _ in range(NPH)]

                def emit_block(wb, n, lo=0, hi=None):
                    for g0 in kwaves[wb][lo:hi]:
                        mov = adj_pair(g0, n)
                        for m in range(mt):
                            nc.tensor.matmul(
                                ps_zs[n][m][:],
                                h_read(m, g0),
                                mov,
                                start=(mm_cnt[n][m] == 0),
                                stop=(mm_cnt[n][m] == KP - 1),
                                perf_mode=DR,
                            )
                            mm_cnt[n][m] += 1

                def emit_block_pair(wb):
                    # both phases per stationary H-pair: the second matmul
                    # reuses the just-loaded weights (halves LDWEIGHTS)
                    for g0 in kwaves[wb]:
                        for m in range(mt):
                            h = h_read(m, g0)
                            for n in range(NPH):
                                nc.tensor.matmul(
                                    ps_zs[n][m][:],
                                    h,
                                    adj_pair(g0, n),
                                    start=(mm_cnt[n][m] == 0),
                                    stop=(mm_cnt[n][m] == KP - 1),
                                    perf_mode=DR,
                                )
                                mm_cnt[n][m] += 1

                def emit_epilogue(n):
                    zt_p = []
                    for m in range(mt):
                        if last:
                            o_st = ostage.tile([128, PH], F32, tag="ost")
                            nc.scalar.activation(
                                o_st[:], ps_zs[n][m][:], RELU,
                                bias=bias_cur[:, m : m + 1], scale=SL[li],
                            )
                            nc.sync.dma_start(
                                outT[m * 128 : (m + 1) * 128,
                                     n * PH : (n + 1) * PH],
                                o_st[:],
                            )
                        else:
                            z_t = ztp.tile([128, PH], BF16, tag="zt",
                                           name=f"z{li + 1}_{m}_{n}")
                            nc.scalar.activation(
                                z_t[:], ps_zs[n][m][:], RELU,
                                bias=bias_cur[:, m : m + 1], scale=SL[li],
                            )
                            zt_p.append(z_t)
                    if last:
                        return
                    # negated z column-sums (moving operands of dcol's
                    # colsum(H) part): ncz[kx] = -sum_rows z[:, kx]
                    ncz = []
                    for kx in range(kxn2):
                        czf = czp.tile([128, 1], F32, tag="czf")
                        nc.vector.tensor_reduce(
                            czf[:], zt_p[kx][:], mybir.AxisListType.X,
                            mybir.AluOpType.add, negate=True,
                        )
                        czt = czp.tile([128, 1], BF16, tag="cz")
                        nc.vector.tensor_copy(czt[:], czf[:])
                        ncz.append(czt)
                    # XW(l+1) for this phase's rows -> bounce -> AllGather.
                    # Phase 1's bounce carries a 4-row f32 tail: the layer's
                    # combined dcol partial rides that H gather, so no
                    # separate AllReduce sits on the CC queue.
                    tail = 4 if n == 1 else 0
                    bounce = dram.tile([PH + tail, do2], FP8,
                                       tag=f"hb{li}_{n}", name=f"hb{li}_{n}")
                    h_sts = []
                    for j in range(HALF):
                        ps_h = psh.tile([128, do2], F32, tag="psh")
                        for kx in range(kxn2):
                            nc.tensor.matmul(
                                ps_h[:],
                                zt_p[kx][:, j * 128 : (j + 1) * 128],
                                w_sb[li + 1][:, kx, :],
                                start=(kx == 0),
                                stop=(kx == kxn2 - 1),
                            )
                        h_st = hstage.tile([128, do2], FP8, tag="hst")
                        nc.vector.tensor_copy(h_st[:], ps_h[:])
                        nc.sync.dma_start(
                            bounce[j * 128 : (j + 1) * 128, :], h_st[:]
                        )
                        h_sts.append(h_st)
                    # this phase's dcol partial row: colsum(Hq) - colsum_z@W
                    # (ones / ncz stationary, fat moving operands)
                    ps_cs = psh.tile([128, do2], F32, tag="psh")
                    for j in range(HALF):
                        nc.tensor.matmul(
                            ps_cs[0:1, :], ones_sb[:], h_sts[j][:],
                            start=(j == 0), stop=False,
                        )
                    for kx in range(kxn2):
                        nc.tensor.matmul(
                            ps_cs[0:1, :], ncz[kx][:], w_sb[li + 1][:, kx, :],
                            start=False, stop=(kx == kxn2 - 1),
                        )
                    dcr = dcs_p.tile([1, do2], F32, tag="dcr",
                                     name=f"dcr{li}_{n}")
                    nc.vector.tensor_copy(dcr[:], ps_cs[0:1, :])
                    if n == 0:
                        emit_epilogue.dcr0 = dcr
                    else:
                        dsum = dcs_p.tile([1, do2], F32, tag="dcr",
                                          name=f"dsum{li}")
                        nc.vector.tensor_tensor(
                            dsum[:], emit_epilogue.dcr0[:], dcr[:],
                            mybir.AluOpType.add,
                        )
                        nc.sync.dma_start(
                            bounce[PH : PH + 4, :].bitcast(F32), dsum[:]
                        )
                    gath = dram.tile(
                        [NCORES * (PH + tail), do2], FP8, addr_space="Shared",
                        tag=f"hg{li}_{n}", name=f"hg{li}_{n}",
                    )
                    nc.gpsimd.collective_compute(
                        "AllGather",
                        mybir.AluOpType.bypass,
                        ins=[bounce[:].opt()],
                        outs=[gath[:].opt()],
                        replica_groups=[list(range(NCORES))],
                    )
                    for ci in range(len(h_next)):
                        c0 = ci * 256
                        dc = min(256, do2 - c0)
                        for c in range(NCORES):
                            r0 = c * (PH + tail)
                            nc.sync.dma_start(
                                h_next[ci][n][:, c * HALF : (c + 1) * HALF, :],
                                gath[r0 : r0 + PH, c0 : c0 + dc].rearrange(
                                    "(q p) d -> p q d", p=128
                                ),
                            )
                    if n == 1:
                        # land the 8 dcol tails transposed straight into a
                        # per-partition layout, reduce, scale into the bias
                        for c in range(NCORES):
                            r0 = c * (PH + 4) + PH
                            nc.sync.dma_start(
                                tl_t[:, :, c],
                                gath[r0 : r0 + 4, :].bitcast(F32)
                                .rearrange("a b -> (a b)")
                                .rearrange("(m p) -> p m", p=128),
                            )
                        dcg = dcs_p.tile([128, mt2], F32, tag="dcs",
                                         name=f"dcg{li}")
                        nc.vector.tensor_reduce(
                            dcg[:], tl_t[:], mybir.AxisListType.X,
                            mybir.AluOpType.add,
                        )
                        bnext = biasp.tile([128, mt2], F32, tag="bias",
                                           name=f"bias{li + 2}")
                        nc.vector.tensor_scalar_mul(
                            bnext[:], dcg[:], -0.5 * SL[li + 1]
                        )
                        emit_epilogue.bias_next = bnext

                if not last:
                    # wave-0 work (both phases, merged stationaries) first:
                    # the in-order PE queue then has a full phase of
                    # executable matmuls while the wave-1 gather lands; the
                    # wave-1 blocks stay phase-split so epilogue 0 (and its
                    # gather) launches as early as possible
                    emit_block_pair(0)
                    emit_block(1, 0)
                    emit_epilogue(0)
                    emit_block(1, 1)
                    emit_epilogue(1)
                    # keep-warm fillers: dataless matmuls that chew ~14us of
                    # PE while the next layer's wave-0 gather lands.  A PE
                    # idle gap >3.4us re-throttles HAM to K=4/8 and the next
                    # ~10us of real matmuls run at half clock; the filler
                    # also keeps cores in lockstep, cutting collective skew.
                    nfill = {0: 96, 1: 80, 2: 80, 3: 72, 4: 128}[li]
                    fill = psh.tile([128, PH], F32, tag="psh", name=f"fl{li}")
                    for fi in range(nfill):
                        nc.tensor.matmul(
                            fill[:],
                            adj_res[0][:, fi % 16, 0:128],
                            adj_res[1][:, fi % 16, 0:PH],
                            start=(fi == 0),
                            stop=(fi == nfill - 1),
                        )
                else:
                    # last layer: phase-split so phase 0's output DMAs
                    # overlap phase 1's matmuls; the final wave runs m-major
                    # with each m's activation+output DMA inlined so the
                    # store drain overlaps the remaining matmuls
                    emit_block(0, 0)
                    emit_block(1, 0)
                    emit_epilogue(0)
                    emit_block(0, 1)
                    for m in range(mt):
                        for g0 in kwaves[1]:
                            nc.tensor.matmul(
                                ps_zs[1][m][:],
                                h_read(m, g0),
                                adj_pair(g0, 1),
                                start=(mm_cnt[1][m] == 0),
                                stop=(mm_cnt[1][m] == KP - 1),
                                perf_mode=DR,
                            )
                            mm_cnt[1][m] += 1
                        o_st = ostage.tile([128, PH], F32, tag="ost")
                        nc.scalar.activation(
                            o_st[:], ps_zs[1][m][:], RELU,
                            bias=bias_cur[:, m : m + 1], scale=SL[li],
                        )
                        nc.sync.dma_start(
                            outT[m * 128 : (m + 1) * 128, PH : 2 * PH],
                            o_st[:],
                        )

                if not last:
                    h_read = make_reader(h_next, len(h_next) > 1)
                    bias_cur = emit_epilogue.bias_next

    nc.compile()
    return nc


def prepare_in_maps(inputs):
    X = np.asarray(inputs["X"], dtype=np.float32)
    adj = np.asarray(inputs["adj_"], dtype=np.float32)

    xT_full = np.ascontiguousarray(X.T).astype(NP_BF16)
    ws = [
        (np.asarray(inputs[f"W{j + 1}"], np.float32) * BETA[j]).astype(NP_BF16)
        for j in range(6)
    ]
    csxn_full = (
        -X.astype(NP_BF16).astype(np.float32).sum(axis=0, keepdims=True)
    ).astype(NP_BF16).reshape(D_IN, 1)
    adj_s = adj * ASCALE
    in_maps = []
    for i in range(NCORES):
        rows = slice(i * R, (i + 1) * R)
        m = {
            "adjT": np.ascontiguousarray(adj_s[rows, :].T).astype(NP_FP8),
            "xT": xT_full,
            "csxn": csxn_full,
        }
        for j in range(6):
            m[f"W{j + 1}"] = ws[j]
        in_maps.append(m)
    return in_maps


def kernel(**inputs):
    if "nc" not in _CACHED:
        _CACHED["nc"] = _build()
    nc = _CACHED["nc"]

    in_maps = prepare_in_maps(inputs)
    res = run_bass_kernel_spmd(nc, in_maps, core_ids=list(range(NCORES)))
    out = np.concatenate(
        [np.asarray(r["outT"], dtype=np.float32).T for r in res.results], axis=0
    )
    return out
